# revision 51
# baseline (speedup 1.0000x reference)
"""Trainium2 Bass kernel v2 for nn_AvgModel (AvgResNet2 GNN, B=4 N=8192 D=128).

Per sub-layer, PSUM holds P = x + 1 directly (main matmul + a K=1 ones-row
matmul adding u_b+1), so elementwise is:
  Act:  E = exp(P - 1)                      (1 op, f16 out)
  DVE:  H = max(min(E, 1), P)  [+ hacc via accum_out]
  H^2:  Pool STT(mult,mult) or Act Square   [qacc via accum_out]
Trunk layers add one DVE STT: Xp += P - 1  (Xp := X + 1 representation).
Stats: plain column reduces of hacc/qacc; rsqrt via Ln+Exp; W-scale early so
PE starts next-layer main matmuls before the bias vector is ready.
"""
import numpy as np

import concourse.bass as bass
import concourse.tile as tile
from concourse import bacc, mybir
import concourse.bass_utils as bass_utils

F32 = mybir.dt.float32
F16 = mybir.dt.float16
AF = mybir.ActivationFunctionType
ALU = mybir.AluOpType

B, N, D, NB = 4, 8192, 128, 15
R = B * N              # 32768
Q = 2048               # column chunk
NCH = R // Q           # 16
HQ = Q // 2            # half-chunk = one 2-bank PSUM tile
CPB = N // Q           # chunks per batch = 4
NK = 2 * NB            # 30 sub-layers
NCORES = 8
EPS = 1e-5

_CACHE = {}


def _build():
    nc = bacc.Bacc("TRN2", target_bir_lowering=False, debug=False,
                   num_devices=NCORES)

    def din(name, shape, dt):
        return nc.dram_tensor(name, list(shape), dt, kind="ExternalInput").ap()

    XFh = din("XFh", [6, R], F16)           # inputs transposed, f16 (host)
    W1h = din("W1h", [6, D], F16)
    B1R = din("B1R", [1, D], F16)           # row (b1 + 1)
    WTh = din("WTh", [NK, D, D], F16)       # W[k][:128,:]
    WBh = din("WBh", [NK, D, D], F16)       # W[k][128:,:]
    PK = din("PK", [D, NK * 8], F32)        # per layer: g1 be1 g2 be2 bias
    IDW = din("IDW", [D, D], F16)           # identity (transpose trick)
    W2h = din("W2h", [D, 120], F16)
    Sh = din("Sh", [3, 120], F16)
    CV = din("CV", [D, 4], F32)             # g2, be2, b2(pad)
    OUT = nc.dram_tensor("OUT", [120, R], F32, kind="ExternalOutput").ap()

    from contextlib import ExitStack
    with tile.TileContext(nc) as tc, ExitStack() as stk:
        sb = stk.enter_context(tc.tile_pool(name="sb", bufs=1))
        wp = stk.enter_context(tc.tile_pool(name="wp", bufs=2))
        ep = stk.enter_context(tc.tile_pool(name="ep", bufs=6))
        sq = stk.enter_context(tc.tile_pool(name="sq", bufs=4))
        io = stk.enter_context(tc.tile_pool(name="io", bufs=2))
        tp = stk.enter_context(tc.tile_pool(name="tp", bufs=2))
        ps = stk.enter_context(tc.tile_pool(name="ps", bufs=4, space="PSUM"))

        # persistent state
        Ht = sb.tile([D, R], F16, tag="H")
        Xpt = sb.tile([D, R], F16, tag="Xp")
        pk_t = sb.tile([D, NK * 8], F32, tag="pk")
        nc.sync.dma_start(pk_t[:], PK[:])
        cv_t = sb.tile([D, 4], F32, tag="cv")
        nc.sync.dma_start(cv_t[:], CV[:])
        w2_t = sb.tile([D, 120], F16, tag="w2")
        nc.sync.dma_start(w2_t[:], W2h[:])
        s_t = sb.tile([3, 120], F16, tag="sel")
        nc.sync.dma_start(s_t[:], Sh[:])
        w1_t = sb.tile([6, D], F16, tag="w1")
        nc.sync.dma_start(w1_t[:], W1h[:])
        b1r_t = sb.tile([1, D], F16, tag="b1r")
        nc.sync.dma_start(b1r_t[:], B1R[:])
        idw_t = sb.tile([D, D], F16, tag="idw")
        nc.sync.dma_start(idw_t[:], IDW[:])
        ones_t = sb.tile([1, 512], F16, tag="ones")
        nc.vector.memset(ones_t[:], 1.0)
        neg1_t = sb.tile([D, 1], F32, tag="neg1")
        nc.vector.memset(neg1_t[:], -1.0)

        def new_accs(zero_hacc=True):
            hacc = tp.tile([D, 2 * NCH], F32, tag="hacc")
            if zero_hacc:
                nc.vector.memset(hacc[:], 0.0)
            qacc = tp.tile([D, NCH], F32, tag="qacc")
            return hacc, qacc

        def emit_h2(c, hacc, qacc, on_act):
            """sum(H^2) for chunk c -> qacc[:, c]."""
            cs = slice(c * Q, (c + 1) * Q)
            if on_act:
                dq = sq.tile([D, Q], F16, tag="sq")
                nc.scalar.activation(dq[:], Ht[:, cs], AF.Square,
                                     accum_out=qacc[:, c:c + 1])
            else:
                dq = sq.tile([D, Q], F16, tag="sq")
                nc.vector.scalar_tensor_tensor(
                    dq[:], Ht[:, cs], 1.0, Ht[:, cs],
                    op0=ALU.mult, op1=ALU.mult,
                    accum_out=qacc[:, c:c + 1])

        # Act-side H^2 ops are deferred one chunk so a Square waiting on the
        # DVE H-STT never head-of-line-blocks the next chunk's Exp
        h2_pend = []

        def h2_flush(n=0):
            while len(h2_pend) > n:
                c, hacc, qacc, on_act = h2_pend.pop(0)
                emit_h2(c, hacc, qacc, on_act)

        def h2_push(c, hacc, qacc, on_act):
            if c == NCH - 1:
                h2_flush(0)
                emit_h2(c, hacc, qacc, on_act)
            elif on_act:
                h2_flush(5)
                h2_pend.append((c, hacc, qacc, True))
            else:
                emit_h2(c, hacc, qacc, False)

        def psum_elem(c, pts, hacc, qacc, h2_act):
            """x+1 in PSUM halves: E=exp(pt-1); H=max(min(E,1),pt)."""
            for h in (0, 1):
                hs = slice(c * Q + h * HQ, c * Q + (h + 1) * HQ)
                et = ep.tile([D, HQ], F16, tag="E")
                nc.scalar.activation(et[:], pts[h][:], AF.Exp, bias=neg1_t[:])
                nc.vector.scalar_tensor_tensor(
                    Ht[:, hs], et[:], 1.0, pts[h][:],
                    op0=ALU.min, op1=ALU.max,
                    accum_out=hacc[:, 2 * c + h:2 * c + h + 1])
            h2_push(c, hacc, qacc, h2_act)

        I32 = mybir.dt.int32

        def rsqrt_nr(v, tag, w):
            """[D,w] rsqrt on DVE: bit-trick init + 2 fused Newton steps."""
            sh = tp.tile([D, w], I32, tag=f"sh{tag}")
            nc.vector.tensor_scalar(sh[:], v.bitcast(I32), 1, None,
                                    ALU.arith_shift_right)
            y0i = tp.tile([D, w], I32, tag=f"y0{tag}")
            nc.vector.tensor_scalar(y0i[:], sh[:], -1, 0x5F3759DF,
                                    ALU.mult, ALU.add)
            y = y0i[:].bitcast(F32)
            for it in range(1):
                t = tp.tile([D, w], F32, tag=f"t{it}{tag}")
                nc.vector.tensor_tensor(t[:], y, y, op=ALU.mult)
                nc.vector.scalar_tensor_tensor(
                    t[:], v, -0.5, t[:], op0=ALU.mult, op1=ALU.mult)
                yn = tp.tile([D, w], F32, tag=f"y{it}{tag}")
                nc.vector.scalar_tensor_tensor(
                    yn[:], t[:], 1.5, y, op0=ALU.add, op1=ALU.mult)
                y = yn[:]
            return yn

        def prefetch_w(k):
            wt = wp.tile([D, D], F16, tag="wt")
            nc.sync.dma_start(wt[:], WTh[k, :, :])
            wb = wp.tile([D, D], F16, tag="wb")
            nc.sync.dma_start(wb[:], WBh[k, :, :])
            return wt, wb

        def stats_chain(k, hacc, qacc, wtb):
            """Returns (wps [D,D] f16, ub4 list of [1,D] f16 rows)."""
            col = lambda j: pk_t[:, k * 8 + j:k * 8 + j + 1]
            g1, be1, g2, be2, bv = (col(0), col(1), col(2), col(3), col(4))
            wt, wb = wtb

            h2_flush()
            # -- critical path to wps: var1 -> rsqrt -> a1 -> W-scale --
            bs4 = tp.tile([D, 4], F32, tag="bs4")
            nc.vector.tensor_reduce(
                bs4[:], hacc[:].rearrange("p (b c) -> p b c", b=4),
                axis=mybir.AxisListType.X, op=ALU.add)
            tot = tp.tile([D, 1], F32, tag="tot")
            nc.vector.tensor_reduce(tot[:], bs4[:], axis=mybir.AxisListType.X,
                                    op=ALU.add)
            qt = tp.tile([D, 1], F32, tag="qt")
            nc.vector.tensor_reduce(qt[:], qacc[:], axis=mybir.AxisListType.X,
                                    op=ALU.add)
            tsq = tp.tile([D, 1], F32, tag="tsq")
            nc.vector.tensor_tensor(tsq[:], tot[:], tot[:], op=ALU.mult)
            m2e = tp.tile([D, 1], F32, tag="m2e")
            nc.vector.tensor_scalar(m2e[:], qt[:], 1.0 / R, EPS,
                                    ALU.mult, ALU.add)
            v1 = tp.tile([D, 1], F32, tag="v1")
            nc.vector.scalar_tensor_tensor(
                v1[:], tsq[:], -1.0 / (float(R) * R), m2e[:],
                op0=ALU.mult, op1=ALU.add)
            s1 = rsqrt_nr(v1[:], "a", 1)
            a1 = tp.tile([D, 1], F32, tag="a1")
            nc.vector.tensor_tensor(a1[:], g1, s1[:], op=ALU.mult)
            # W' = a1 (.) WT -- PE main matmuls unblock here
            wps = wp.tile([D, D], F16, tag="wps")
            nc.vector.tensor_scalar(wps[:], wt[:], a1[:], None, ALU.mult)
            # trunk layers consume PSUM as Xp += P-1, so rows carry u+1;
            # only the last (folded) layer wants bare u in PSUM
            add1 = 0.0 if k == NK - 1 else 1.0
            return wps, lambda up: _chain_part2(
                up, wps, bs4, tot, a1, wb, g2, be1, be2, bv, add1)

        def _chain_part2(up, wps, bs4, tot, a1, wb, g2, be1, be2, bv, add1):
            """ga branch + bias rows: overlaps the PE main matmuls."""
            muH = tp.tile([D, 1], F32, tag="muH")
            nc.vector.tensor_scalar(muH[:], tot[:], 1.0 / R, None, ALU.mult)
            mb = tp.tile([D, 4], F32, tag="mb")
            nc.vector.tensor_scalar(mb[:], bs4[:], 1.0 / N, -1.0,
                                    ALU.mult, ALU.add)
            mu2 = tp.tile([D, 1], F32, tag="mu2")
            nc.vector.tensor_reduce(mu2[:], mb[:], axis=mybir.AxisListType.X,
                                    op=ALU.add)
            nc.vector.tensor_scalar(mu2[:], mu2[:], 0.25, None, ALU.mult)
            mbsq = tp.tile([D, 4], F32, tag="mbsq")
            nc.vector.tensor_tensor(mbsq[:], mb[:], mb[:], op=ALU.mult)
            q2 = tp.tile([D, 1], F32, tag="q2")
            nc.vector.tensor_reduce(q2[:], mbsq[:], axis=mybir.AxisListType.X,
                                    op=ALU.add)
            mu2sq = tp.tile([D, 1], F32, tag="mu2sq")
            nc.vector.tensor_tensor(mu2sq[:], mu2[:], mu2[:], op=ALU.mult)
            q2e = tp.tile([D, 1], F32, tag="q2e")
            nc.vector.tensor_scalar(q2e[:], q2[:], 0.25, EPS,
                                    ALU.mult, ALU.add)
            v2 = tp.tile([D, 1], F32, tag="v2")
            nc.vector.tensor_tensor(v2[:], q2e[:], mu2sq[:], op=ALU.subtract)
            s2 = rsqrt_nr(v2[:], "b", 1)
            a2 = tp.tile([D, 1], F32, tag="a2")
            nc.vector.tensor_tensor(a2[:], g2, s2[:], op=ALU.mult)
            ra1 = tp.tile([D, 1], F32, tag="ra1")
            nc.vector.reciprocal(ra1[:], a1[:])
            tvh = tp.tile([D, 1], F16, tag="tvh")
            nc.vector.scalar_tensor_tensor(
                tvh[:], ra1[:], be1, muH[:], op0=ALU.mult, op1=ALU.subtract)
            gv = tp.tile([D, 4], F32, tag="gv")
            nc.vector.scalar_tensor_tensor(
                gv[:], mb[:], mu2[:], a2[:].broadcast_to((D, 4)),
                op0=ALU.subtract, op1=ALU.mult)
            gvh = tp.tile([D, 4], F16, tag="gvh")
            nc.vector.tensor_scalar(gvh[:], gv[:], be2, None, ALU.add)
            # matvecs: u = wps^T tvh + WB^T gvh + bias
            nc.tensor.matmul(up[:, 0:1], wps[:], tvh[:], start=True, stop=True)
            nc.tensor.matmul(up[:, 1:5], wb[:], gvh[:], start=True, stop=True)
            usb = tp.tile([D, 5], F32, tag="usb")
            nc.vector.tensor_copy(usb[:], up[:, 0:5])
            ub4 = tp.tile([D, 4], F16, tag="ub4")
            # rows carry u+1 (PSUM = x+1; for trunk layers Xp += PSUM-1)
            ut1 = tp.tile([D, 1], F32, tag="ut1")
            nc.vector.scalar_tensor_tensor(
                ut1[:], usb[:, 0:1], add1, bv, op0=ALU.add, op1=ALU.add)
            nc.vector.scalar_tensor_tensor(
                ub4[:], usb[:, 1:5], 1.0, ut1[:].broadcast_to((D, 4)),
                op0=ALU.mult, op1=ALU.add)
            # transpose ub4 -> rows [1, D] per batch via identity matmuls
            # into bank 1 of the same PSUM tile (cols 512..1023)
            ubr = []
            for b in range(4):
                nc.tensor.matmul(up[0:1, 512 + b * D:512 + (b + 1) * D],
                                 ub4[:, b:b + 1], idw_t[:],
                                 start=True, stop=True)
                r = tp.tile([1, D], F16, tag=f"ubr{b}")
                nc.vector.tensor_copy(r[:], up[0:1, 512 + b * D:
                                               512 + (b + 1) * D])
                ubr.append(r)
            return ubr

        def mm_ident(c, pts):
            """Accumulate Xp_old into PSUM; depends only on Xpt, so these
            run on PE during the stats chain."""
            for h in (0, 1):
                for qb in range(2):
                    qs = slice(qb * 512, (qb + 1) * 512)
                    o = c * Q + h * HQ + qb * 512
                    nc.tensor.matmul(pts[h][:, qs], idw_t[:],
                                     Xpt[:, o:o + 512],
                                     start=True, stop=False)

        def mm_mains(c, wA, pts, first):
            for h in (0, 1):
                for qb in range(2):
                    qs = slice(qb * 512, (qb + 1) * 512)
                    o = c * Q + h * HQ + qb * 512
                    nc.tensor.matmul(pts[h][:, qs], wA[:],
                                     Ht[:, o:o + 512],
                                     start=first, stop=False)

        def mm_bias(c, ubr, pts):
            b = c // CPB
            for h in (0, 1):
                for qb in range(2):
                    qs = slice(qb * 512, (qb + 1) * 512)
                    nc.tensor.matmul(
                        pts[h][:, qs], ubr[b][:], ones_t[:],
                        start=False, stop=True)

        def mm_half(c, h, wA, ubr, pts, first, fold):
            b = c // CPB
            o0 = c * Q + h * HQ
            for qb in range(2):
                qs = slice(qb * 512, (qb + 1) * 512)
                if fold:
                    nc.tensor.matmul(pts[h][:, qs], idw_t[:],
                                     Xpt[:, o0 + qb * 512:o0 + (qb + 1) * 512],
                                     start=True, stop=False)
            for qb in range(2):
                qs = slice(qb * 512, (qb + 1) * 512)
                nc.tensor.matmul(pts[h][:, qs], wA[:],
                                 Ht[:, o0 + qb * 512:o0 + (qb + 1) * 512],
                                 start=first, stop=False)
            for qb in range(2):
                qs = slice(qb * 512, (qb + 1) * 512)
                nc.tensor.matmul(pts[h][:, qs], ubr[b][:], ones_t[:],
                                 start=False, stop=True)

        def wb_xp(c, pts):
            for h in (0, 1):
                hs = slice(c * Q + h * HQ, c * Q + (h + 1) * HQ)
                nc.scalar.activation(Xpt[:, hs], pts[h][:], AF.Identity)

        # ---- conv1: PSUM = W1^T xfh + (b1+1) = Xp0; elementwise from PSUM
        wnext = prefetch_w(0)
        hacc, qacc = new_accs()
        for c in range(NCH):
            xfh = io.tile([6, Q], F16, tag="xf")
            nc.sync.dma_start(xfh[:], XFh[:, c * Q:(c + 1) * Q])
            ptA = ps.tile([D, HQ], F32, tag="x")
            ptB = ps.tile([D, HQ], F32, tag="x")
            pts = (ptA, ptB)
            for h in (0, 1):
                for qb in range(2):
                    qs = slice(qb * 512, (qb + 1) * 512)
                    o = h * HQ + qb * 512
                    nc.tensor.matmul(pts[h][:, qs], w1_t[:],
                                     xfh[:, o:o + 512],
                                     start=True, stop=False)
                    nc.tensor.matmul(pts[h][:, qs], b1r_t[:], ones_t[:],
                                     start=False, stop=True)
            psum_elem(c, pts, hacc, qacc, h2_act=(c % 5 == 0))
            wb_xp(c, pts)

        # trunk H-assembly lags its Exp by one chunk so the DVE queue
        # never stalls on Act: while Act computes Exp_c, DVE runs the next
        # chunk's X-STTs, then assembles H_{c-1} whose E is long ready
        trunk_pend = []

        def trunk_flush(n=0):
            while len(trunk_pend) > n:
                c, et, hacc, qacc = trunk_pend.pop(0)
                cs = slice(c * Q, (c + 1) * Q)
                nc.vector.scalar_tensor_tensor(
                    Ht[:, cs], et[:], 1.0, Xpt[:, cs],
                    op0=ALU.min, op1=ALU.max,
                    accum_out=hacc[:, 2 * c:2 * c + 1])
                h2_push(c, hacc, qacc, True)

        def trunk_elem(c, hacc, qacc):
            """Trunk Xp in SBUF: E=exp(Xp-1) inline; H-STT deferred."""
            cs = slice(c * Q, (c + 1) * Q)
            et = ep.tile([D, Q], F16, tag="E")
            nc.scalar.activation(et[:], Xpt[:, cs], AF.Exp, bias=neg1_t[:])
            trunk_pend.append((c, et, hacc, qacc))
            if c == NCH - 1:
                trunk_flush(0)
            else:
                trunk_flush(3)

        # ---- 30 sub-layers ----
        # even k: PSUM = W'H + (u+1) = x_int + 1 (interior, no trunk write)
        # odd k:  DVE folds Xp += PSUM - 1, elementwise reads Xp from SBUF
        # last k: PE folds Xp_old into PSUM (= X_fin + 1) -- no extra pass
        def do_elem(k, c, pts, hacc, qacc, last):
            if k % 2 == 0:
                psum_elem(c, pts, hacc, qacc, h2_act=((c % 8) < 5))
            elif last:
                psum_elem(c, pts, hacc, qacc, h2_act=(c % 6 == 0))
            else:
                for h in (0, 1):
                    hs = slice(c * Q + h * HQ, c * Q + (h + 1) * HQ)
                    nc.vector.scalar_tensor_tensor(
                        Xpt[:, hs], pts[h][:], -1.0, Xpt[:, hs],
                        op0=ALU.add, op1=ALU.add)
                trunk_elem(c, hacc, qacc)

        for k in range(NK):
            wps, finish = stats_chain(k, hacc, qacc, wnext)
            if k + 1 < NK:
                wnext = prefetch_w(k + 1)
            last = (k == NK - 1)
            hacc, qacc = new_accs(zero_hacc=(k % 2 == 1 and not last))
            # chunk 0 mains go to the PE queue before the chain's matvec
            # matmuls; `up` is allocated first so chunk 1's tile reuses its
            # (short-lived) slot
            up = ps.tile([D, HQ], F32, tag="x")
            pt0A = ps.tile([D, HQ], F32, tag="x")
            pt0B = ps.tile([D, HQ], F32, tag="x")
            pt0 = (pt0A, pt0B)
            if last:
                mm_ident(0, pt0)
            mm_mains(0, wps, pt0, first=not last)
            ubr = finish(up)
            mm_bias(0, ubr, pt0)
            do_elem(k, 0, pt0, hacc, qacc, last)
            for c in range(1, NCH):
                ptA = ps.tile([D, HQ], F32, tag="x")
                ptB = ps.tile([D, HQ], F32, tag="x")
                pts = (ptA, ptB)
                mm_half(c, 0, wps, ubr, pts, not last, last)
                mm_half(c, 1, wps, ubr, pts, not last, last)
                do_elem(k, c, pts, hacc, qacc, last)

        # ---- conv2: BN(128) -> W2 + b2 + selector ----
        h2_flush()
        g2c, be2c, b2c = cv_t[:, 0:1], cv_t[:, 1:2], cv_t[:, 2:3]
        tot = tp.tile([D, 1], F32, tag="tot")
        nc.vector.tensor_reduce(tot[:], hacc[:], axis=mybir.AxisListType.X,
                                op=ALU.add)
        qt = tp.tile([D, 1], F32, tag="qt")
        nc.vector.tensor_reduce(qt[:], qacc[:], axis=mybir.AxisListType.X,
                                op=ALU.add)
        muH = tp.tile([D, 1], F32, tag="muH")
        nc.vector.tensor_scalar(muH[:], tot[:], 1.0 / R, None, ALU.mult)
        m2 = tp.tile([D, 1], F32, tag="m2")
        nc.vector.tensor_scalar(m2[:], qt[:], 1.0 / R, None, ALU.mult)
        musq = tp.tile([D, 1], F32, tag="musq")
        nc.vector.tensor_tensor(musq[:], muH[:], muH[:], op=ALU.mult)
        vf = tp.tile([D, 1], F32, tag="vf")
        nc.vector.scalar_tensor_tensor(
            vf[:], m2[:], EPS, musq[:], op0=ALU.add, op1=ALU.subtract)
        s12 = rsqrt_nr(vf[:], "f", 1)
        af = tp.tile([D, 1], F32, tag="af")
        nc.vector.tensor_tensor(af[:], g2c, s12[:, 0:1], op=ALU.mult)
        w2p = wp.tile([D, 120], F16, tag="w2p")
        nc.vector.tensor_scalar(w2p[:], w2_t[:], af[:], None, ALU.mult)
        raf = tp.tile([D, 1], F32, tag="raf")
        nc.vector.reciprocal(raf[:], af[:])
        tvfh = tp.tile([D, 1], F16, tag="tvfh")
        nc.vector.scalar_tensor_tensor(
            tvfh[:], raf[:], be2c, muH[:], op0=ALU.mult, op1=ALU.subtract)
        upf = ps.tile([D, HQ], F32, tag="x")
        nc.tensor.matmul(upf[0:120, 0:1], w2p[:], tvfh[:],
                         start=True, stop=True)
        ufsb = tp.tile([D, 1], F32, tag="ufsb")
        nc.vector.tensor_tensor(ufsb[0:120, :], upf[0:120, 0:1],
                                b2c[0:120, :], op=ALU.add)
        for c in range(NCH):
            xf3 = io.tile([3, Q], F16, tag="xf3")
            nc.sync.dma_start(xf3[:], XFh[3:6, c * Q:(c + 1) * Q])
            for h in (0, 1):
                pt = ps.tile([D, HQ], F32, tag="x")
                for qb in range(2):
                    qs = slice(qb * 512, (qb + 1) * 512)
                    o = c * Q + h * HQ + qb * 512
                    nc.tensor.matmul(pt[0:120, qs], w2p[:],
                                     Ht[:, o:o + 512],
                                     start=True, stop=False)
                    nc.tensor.matmul(
                        pt[0:120, qs], s_t[:],
                        xf3[:, h * HQ + qb * 512:h * HQ + (qb + 1) * 512],
                        start=False, stop=True)
                ot = io.tile([120, HQ], F32, tag="ot")
                nc.scalar.activation(ot[:], pt[0:120, :], AF.Identity,
                                     bias=ufsb[0:120, :])
                nc.sync.dma_start(
                    OUT[:, c * Q + h * HQ:c * Q + (h + 1) * HQ], ot[:])

    nc.compile()
    return nc


def _prep(inputs):
    inp = np.asarray(inputs["inputs"], np.float32)          # [B, N, 6]
    rn_W = np.asarray(inputs["rn_W"], np.float32)           # [NB,2,256,128]
    rn_g = np.asarray(inputs["rn_gamma"], np.float32)
    rn_b = np.asarray(inputs["rn_beta"], np.float32)
    rn_bias = np.asarray(inputs["rn_b"], np.float32)        # [NB,2,128]
    XFa = np.ascontiguousarray(inp.reshape(R, 6).T).astype(np.float16)
    W1a = np.asarray(inputs["W1"], np.float32).astype(np.float16)
    B1a = (np.asarray(inputs["b1"], np.float32) + 1.0).reshape(1, D)
    WT = rn_W[:, :, :D, :].reshape(NK, D, D).astype(np.float16)
    WB = rn_W[:, :, D:, :].reshape(NK, D, D).astype(np.float16)
    PKa = np.zeros((D, NK * 8), np.float32)
    for kk in range(NK):
        l, j = kk // 2, kk % 2
        PKa[:, kk * 8 + 0] = rn_g[l, j, :D]
        PKa[:, kk * 8 + 1] = rn_b[l, j, :D]
        PKa[:, kk * 8 + 2] = rn_g[l, j, D:]
        PKa[:, kk * 8 + 3] = rn_b[l, j, D:]
        PKa[:, kk * 8 + 4] = rn_bias[l, j]
    W2a = np.asarray(inputs["W2"], np.float32).astype(np.float16)
    Sa = np.zeros((3, 120), np.float16)
    for f in range(120):
        Sa[f % 3, f] = 1.0
    CVa = np.zeros((D, 4), np.float32)
    CVa[:, 0] = np.asarray(inputs["g2"], np.float32)
    CVa[:, 1] = np.asarray(inputs["be2"], np.float32)
    CVa[:120, 2] = np.asarray(inputs["b2"], np.float32)
    IDa = np.eye(D).astype(np.float16)
    return {"XFh": XFa, "W1h": W1a, "B1R": B1a.astype(np.float16),
            "WTh": WT, "WBh": WB, "PK": PKa, "IDW": IDa,
            "W2h": W2a, "Sh": Sa, "CV": CVa}


def _ref_numpy(inputs):
    """Exact fallback (unused for the spec'd all-ones mask)."""
    mask = np.asarray(inputs["mask"], np.float32)
    x = np.asarray(inputs["inputs"], np.float32)
    W1 = inputs["W1"]; b1 = inputs["b1"]
    x = x @ W1 + b1

    def gbn(t, g, b):
        mu = t.mean((0, 1)); v = ((t - mu) ** 2).mean((0, 1))
        return (t - mu) / np.sqrt(v + EPS) * g + b

    def gavg(t):
        return (t * mask).sum(1, keepdims=True) / mask.sum(1, keepdims=True)

    for l in range(NB):
        res = x
        for j in range(2):
            h = np.where(x > 0, x, np.expm1(np.minimum(x, 0)))
            ga = np.broadcast_to(gavg(h), h.shape)
            h = np.concatenate([h, ga], 2)
            h = gbn(h, inputs["rn_gamma"][l, j], inputs["rn_beta"][l, j])
            x = h @ inputs["rn_W"][l, j] + inputs["rn_b"][l, j]
        x = x + res
    h = np.where(x > 0, x, np.expm1(np.minimum(x, 0)))
    x = gbn(h, inputs["g2"], inputs["be2"]) @ inputs["W2"] + inputs["b2"]
    return (x + np.tile(np.asarray(inputs["inputs"])[:, :, -3:], (1, 1, 40))
            ).astype(np.float32)


def kernel(**inputs):
    mask = np.asarray(inputs["mask"], np.float32)
    if not (np.all(mask == 1.0) and np.asarray(inputs["inputs"]).shape ==
            (B, N, 6)):
        return _ref_numpy(inputs)
    if "nc" not in _CACHE:
        _CACHE["nc"] = _build()
    nc = _CACHE["nc"]
    im = _prep(inputs)
    res = bass_utils.run_bass_kernel_spmd(
        nc, [im] * NCORES, core_ids=list(range(NCORES)))
    out = res.results[0]["OUT"]                      # [120, R]
    return np.ascontiguousarray(out.T).reshape(B, N, 120).astype(np.float32)
